# revision 9
# baseline (speedup 1.0000x reference)
"""BitLinear (BitNet b1.58) forward kernel for Trainium2, 8 NeuronCores.

y = act_quant(x) @ weight_quant(W)^T + bias
  - activation quant: per-token absmax int8 fake-quant (values in [-127,127])
  - weight quant: per-tensor mean-absmax ternary fake-quant {-1,0,1}

Sharding: data-parallel over the batch dim (8 batches -> 1 per core);
W and bias are replicated per core, each core computes mean(|W|) locally
(no collectives needed).

Numerics (rel err ~3.3e-3 vs fp32 reference, gate is 2e-2):
  * q = round(x * s) with s = 127*recip(mx) gives exact int8 values; they are
    pre-scaled by c_tok = mx*mean|W|/127 and rounded once to bf16 (qs), so
    PSUM accumulates y - bias directly and the epilogue is a pure
    bias-add + bf16 downcast. The ternary t in {-1,0,1} is exact in bf16.
  * mean(|W|) must match the reference's fp32 value to ~2e-7 relative (nearest
    weight sits 2.6e-7 from a ternary rounding boundary), so the reduction
    uses an exact hi/lo split summation.
  * y is stored bf16 and upcast on the host.

Engine layout per token tile (steady state, software-pipelined by 1 tile):
  PE     : ONLY the 16 N=512 bf16 matmuls (3.4us -> the bottleneck)
  DMAxbar: qs transpose [128,512]->[128,4,128] (no PE transposes at all)
  ACT    : r1 = x*sx+MAGIC; epilogue copy of half B PSUM->SBUF
  DVE    : absmax, recip smalls, qs = (r1-MAGIC)*c_tok -> bf16,
           epilogue half A = PSUM + bias -> bf16
  GpSimd : epilogue part B = +bias -> bf16 (SBUF only; GpSimd can't see PSUM)
  sync Q : x-load triggers ONLY (deep prefetch, no head-of-line blockers)
  scalar Q: qs-xbar triggers (fire right after qs, short wait)
  gpsimd Q: y-store triggers
"""

import os
import sys

import numpy as np

B, S, DIN, DOUT = 8, 4096, 512, 2048
N_CORES = 8

MAGIC = 12582912.0  # 1.5 * 2^23: (v + MAGIC) - MAGIC == round-half-even(v), |v| < 2^22
C_GRID_11 = 6144.0  # 1.5 * 2^12: rounds to multiples of 2^-11 (values <= ~26)
C_GRID_4 = 786432.0  # 1.5 * 2^19: rounds to multiples of 2^-4  (values <= ~400)
EPS = 1e-6

_cached = {}


def _ensure_path():
    try:
        import concourse  # noqa: F401
    except ImportError:
        for p in ("/opt/trn_rl_repo", os.path.expanduser("~/.axon_site/_ro/trn_rl_repo")):
            if os.path.isdir(p) and p not in sys.path:
                sys.path.insert(0, p)


def build_program(s_tiles=S // 128):
    """Emit the Bass/Tile program for one core: x [s_tiles*128, DIN] -> y."""
    _ensure_path()
    from contextlib import ExitStack

    import concourse.bacc as bacc
    import concourse.tile as tile
    from concourse import mybir

    f32 = mybir.dt.float32
    bf16 = mybir.dt.bfloat16
    Alu = mybir.AluOpType
    X = mybir.AxisListType.X
    Copy = mybir.ActivationFunctionType.Copy
    SROWS = s_tiles * 128

    nc = bacc.Bacc("TRN2", target_bir_lowering=False, debug=False, num_devices=N_CORES)
    x_d = nc.dram_tensor("x", [SROWS, DIN], f32, kind="ExternalInput").ap()
    w_d = nc.dram_tensor("w", [DOUT, DIN], f32, kind="ExternalInput").ap()
    b_d = nc.dram_tensor("bias", [1, DOUT], f32, kind="ExternalInput").ap()
    y_d = nc.dram_tensor("y", [SROWS, DOUT], bf16, kind="ExternalOutput").ap()

    KC = DIN // 128  # 4 contraction chunks
    OC = DOUT // 128  # 16 output chunks
    HALF = 1280  # epilogue split: DVE takes [0:1280], ACT+GpSimd take [1280:2048]

    with tile.TileContext(nc) as tc, ExitStack() as ctx:
        cpool = ctx.enter_context(tc.tile_pool(name="const", bufs=1))
        wallp = ctx.enter_context(tc.tile_pool(name="wall", bufs=1))
        wtmpp = ctx.enter_context(tc.tile_pool(name="wtmp", bufs=3))
        wqp = ctx.enter_context(tc.tile_pool(name="wq", bufs=3))
        statp = ctx.enter_context(tc.tile_pool(name="stat", bufs=1))
        tTp = ctx.enter_context(tc.tile_pool(name="tT", bufs=1))
        xp = ctx.enter_context(tc.tile_pool(name="x", bufs=8))
        r1p = ctx.enter_context(tc.tile_pool(name="r1", bufs=3))
        qp = ctx.enter_context(tc.tile_pool(name="q", bufs=3))
        qtp = ctx.enter_context(tc.tile_pool(name="qt", bufs=4))
        mxp = ctx.enter_context(tc.tile_pool(name="mx", bufs=16))
        yfp = ctx.enter_context(tc.tile_pool(name="yf", bufs=3))
        yp = ctx.enter_context(tc.tile_pool(name="y", bufs=3))
        py = ctx.enter_context(tc.tile_pool(name="py", bufs=2, space="PSUM"))

        # ---- W load first: per-chunk DMAs so abs-sums start immediately, and
        # nothing else sits ahead of them in the sync HWDGE FIFO ----
        w_all = wallp.tile([128, OC, DIN], f32)
        w_r = w_d.rearrange("(c p) d -> p c d", p=128)
        for c in range(OC):
            nc.sync.dma_start(w_all[:, c : c + 1, :], w_r[:, c : c + 1, :])

        # ---- constants ----
        b_row = cpool.tile([1, DOUT], f32)
        nc.scalar.dma_start(b_row[:], b_d)
        bias_bc = cpool.tile([128, DOUT], f32)
        nc.gpsimd.dma_start(bias_bc[:], b_d.broadcast_to([128, DOUT]))
        ones128 = cpool.tile([128, 128], f32)
        nc.vector.memset(ones128[:], 1.0)

        # ---- mean(|W|): exact-split summation ----
        wsum = statp.tile([128, OC], f32)
        for c in range(OC):
            nc.vector.tensor_reduce(
                wsum[:, c : c + 1], w_all[:, c, :],
                axis=X, op=Alu.add, apply_absolute_value=True,
            )
        # split per-chunk sums (<= ~26) to a 2^-11 grid -> exact 16-way add
        hh = statp.tile([128, OC], f32)
        ll = statp.tile([128, OC], f32)
        nc.vector.tensor_scalar(hh[:], wsum[:], C_GRID_11, C_GRID_11, op0=Alu.add, op1=Alu.subtract)
        nc.vector.tensor_tensor(ll[:], wsum[:], hh[:], op=Alu.subtract)
        hs = statp.tile([128, 1], f32)
        ls = statp.tile([128, 1], f32)
        nc.vector.tensor_reduce(hs[:], hh[:], axis=X, op=Alu.add)
        nc.vector.tensor_reduce(ls[:], ll[:], axis=X, op=Alu.add)
        # split per-partition totals (<= ~400) to a 2^-4 grid -> exact 128-way add
        red = statp.tile([128, 2], f32)
        l2 = statp.tile([128, 1], f32)
        nc.vector.tensor_scalar(red[:, 0:1], hs[:], C_GRID_4, C_GRID_4, op0=Alu.add, op1=Alu.subtract)
        nc.vector.tensor_tensor(l2[:], hs[:], red[:, 0:1], op=Alu.subtract)
        nc.vector.tensor_tensor(red[:, 1:2], l2[:], ls[:], op=Alu.add)
        # cross-partition sum + broadcast in one exact fp32 ones-matmul:
        # out[m, j] = sum_p red[p, j] for every m
        pred = py.tile([128, DOUT], f32, tag="ytile", name="pred")
        nc.tensor.matmul(pred[:, 0:2], ones128[:], red[:], start=True, stop=True)
        redo = statp.tile([128, 2], f32)
        nc.scalar.copy(redo[:], pred[:, 0:2])
        ssum = statp.tile([128, 1], f32)
        nc.vector.tensor_tensor(ssum[:], redo[:, 0:1], redo[:, 1:2], op=Alu.add)
        mean_t = statp.tile([128, 1], f32)
        nc.vector.tensor_scalar(mean_t[:], ssum[:], 1.0 / (DOUT * DIN), None, op0=Alu.mult)
        nc.vector.tensor_scalar(mean_t[:], mean_t[:], EPS, None, op0=Alu.max)
        s_w = statp.tile([128, 1], f32)  # 1/mean: the quantization scale
        nc.vector.reciprocal(s_w[:], mean_t[:])
        v_w = statp.tile([128, 1], f32)  # fl(1/s_w): the dequant magnitude (matches ref)
        nc.vector.reciprocal(v_w[:], s_w[:])
        vw127 = statp.tile([128, 1], f32)  # v_w / 127, folded once into the qs scale
        nc.vector.tensor_scalar(vw127[:], v_w[:], 1.0 / 127.0, None, op0=Alu.mult)

        # ---- W quantize; transpose on the DMA xbar: tT[:, k, o] = t[o, k*128+p] ----
        tT = tTp.tile([128, KC, DOUT], bf16)
        for c in range(OC):
            wc = w_all[:, c, :]
            wr1 = wtmpp.tile([128, DIN], f32, tag="wr1")
            nc.scalar.activation(wr1[:], wc, Copy, bias=MAGIC, scale=s_w[:])
            wr2 = wtmpp.tile([128, DIN], f32, tag="wr2")
            nc.vector.tensor_scalar(wr2[:], wr1[:], MAGIC, 1.0, op0=Alu.subtract, op1=Alu.min)
            wq = wqp.tile([128, DIN], bf16)
            nc.vector.tensor_scalar(wq[:], wr2[:], -1.0, None, op0=Alu.max)
            nc.scalar.dma_start_transpose(tT[:, :, c * 128 : (c + 1) * 128], wq[:])

        # ---- main loop over token tiles (epilogue software-pipelined by 1) ----
        state = {}

        def epilogue(j, ph, c_tok):
            ysb = yp.tile([128, DOUT], bf16)
            # part A: DVE adds bias straight out of PSUM, downcasts to bf16
            nc.vector.tensor_tensor(ysb[:, 0:HALF], ph[:, 0:HALF], bias_bc[:, 0:HALF], op=Alu.add)
            # part B: ACT stages PSUM->SBUF, GpSimd adds bias (it can't see PSUM)
            yf = yfp.tile([128, DOUT - HALF], f32)
            nc.scalar.copy(yf[:], ph[:, HALF:DOUT])
            nc.gpsimd.tensor_tensor(ysb[:, HALF:DOUT], yf[:], bias_bc[:, HALF:DOUT], op=Alu.add)
            nc.gpsimd.dma_start(y_d[j * 128 : (j + 1) * 128, :], ysb[:])

        for i in range(s_tiles):
            xt = xp.tile([128, DIN], f32)
            nc.sync.dma_start(xt[:], x_d[i * 128 : (i + 1) * 128, :])

            mx = mxp.tile([128, 1], f32, tag="mx")
            nc.vector.tensor_reduce(mx[:], xt[:], axis=X, op=Alu.max, apply_absolute_value=True)
            sx = mxp.tile([128, 1], f32, tag="sx")
            nc.vector.reciprocal(sx[:], mx[:])
            nc.vector.tensor_scalar(sx[:], sx[:], 127.0, None, op0=Alu.mult)
            c_tok = mxp.tile([128, 1], f32, tag="ct")
            nc.vector.tensor_tensor(c_tok[:], mx[:], vw127[:], op=Alu.mult)

            r1 = r1p.tile([128, DIN], f32)
            nc.scalar.activation(r1[:], xt[:], Copy, bias=MAGIC, scale=sx[:])
            # qs = (round(x*sx)) * c_tok, rounded once to bf16
            qs = qp.tile([128, DIN], bf16)
            nc.vector.tensor_scalar(qs[:], r1[:], MAGIC, c_tok[:], op0=Alu.subtract, op1=Alu.mult)

            qT = qtp.tile([128, KC, 128], bf16)
            nc.scalar.dma_start_transpose(qT[:], qs[:])

            ph = py.tile([128, DOUT], f32, tag="ytile", name="ph")
            for k in range(KC):
                lhsT = qT[:, k, :]
                for n in range(KC):
                    nc.tensor.matmul(
                        ph[:, n * 512 : (n + 1) * 512], lhsT,
                        tT[:, k, n * 512 : (n + 1) * 512],
                        start=(k == 0), stop=(k == KC - 1),
                    )

            if i > 0:
                epilogue(i - 1, state["ph"], state["ct"])
            state = {"ph": ph, "ct": c_tok}

        epilogue(s_tiles - 1, state["ph"], state["ct"])

    nc.compile()
    return nc


def _get_program():
    if "nc" not in _cached:
        _cached["nc"] = build_program()
    return _cached["nc"]


def kernel(x: np.ndarray, weight: np.ndarray, bias: np.ndarray) -> np.ndarray:
    _ensure_path()
    from concourse.bass_utils import run_bass_kernel_spmd

    x = np.ascontiguousarray(x, dtype=np.float32)
    weight = np.ascontiguousarray(weight, dtype=np.float32)
    bias2d = np.ascontiguousarray(bias, dtype=np.float32).reshape(1, DOUT)

    nc = _get_program()
    in_maps = [
        {"x": x[c], "w": weight, "bias": bias2d} for c in range(N_CORES)
    ]
    res = run_bass_kernel_spmd(nc, in_maps, core_ids=list(range(N_CORES)))
    _cached["last_results"] = res
    y = np.stack(
        [np.asarray(res.results[c]["y"]).astype(np.float32) for c in range(N_CORES)],
        axis=0,
    )
    return y


# revision 10
# speedup vs baseline: 1.4915x; 1.4915x over previous
"""BitLinear (BitNet b1.58) forward kernel for Trainium2, 8 NeuronCores.

y = act_quant(x) @ weight_quant(W)^T + bias
  - activation quant: per-token absmax int8 fake-quant (values in [-127,127])
  - weight quant: per-tensor mean-absmax ternary fake-quant {-1,0,1}

Sharding: data-parallel over the batch dim (8 batches -> 1 per core);
W and bias are replicated per core, each core computes mean(|W|) locally
(no collectives needed).

Numerics (rel err ~3.4e-3 vs fp32 reference, gate is 2e-2):
  * q = round(x * s) with s = 127*recip(mx) gives exact int8 values; they are
    pre-scaled by c_tok = mx*mean|W|/127 and rounded once to bf16 (qs), so
    PSUM accumulates y - bias directly and the epilogue is a pure
    bias-add + bf16 downcast. The ternary t in {-1,0,1} is exact in bf16.
  * mean(|W|) must match the reference's fp32 value to ~2e-7 relative (nearest
    weight sits 2.6e-7 from a ternary rounding boundary), so the reduction
    uses an exact hi/lo split summation.
  * y is stored bf16 and upcast on the host.

Engine layout per token tile (steady state, software-pipelined by 1 tile):
  PE     : 16 N=512 bf16 matmuls + 4 128x128 transposes of qs (the PE xbar
           DMA-transpose path is poison: the Tile scheduler serializes it
           against all in-flight DMA queues, stalling the pipeline)
  ACT    : r1 = x*sx+MAGIC; epilogue copy of half B PSUM->SBUF
  DVE    : absmax, recip smalls, qs = (r1-MAGIC)*c_tok -> bf16, qT copy
           PSUM->SBUF, epilogue half A = PSUM + bias -> bf16
  GpSimd : epilogue half B = +bias -> bf16 (GpSimd cannot touch PSUM)
  sync Q : x-load triggers ONLY (deep prefetch, no head-of-line blockers)
  gpsimd Q: y-store triggers
"""

import os
import sys

import numpy as np

B, S, DIN, DOUT = 8, 4096, 512, 2048
N_CORES = 8

MAGIC = 12582912.0  # 1.5 * 2^23: (v + MAGIC) - MAGIC == round-half-even(v), |v| < 2^22
C_GRID_11 = 6144.0  # 1.5 * 2^12: rounds to multiples of 2^-11 (values <= ~26)
C_GRID_4 = 786432.0  # 1.5 * 2^19: rounds to multiples of 2^-4  (values <= ~400)
EPS = 1e-6

_cached = {}


def _ensure_path():
    try:
        import concourse  # noqa: F401
    except ImportError:
        for p in ("/opt/trn_rl_repo", os.path.expanduser("~/.axon_site/_ro/trn_rl_repo")):
            if os.path.isdir(p) and p not in sys.path:
                sys.path.insert(0, p)


def build_program(s_tiles=S // 128):
    """Emit the Bass/Tile program for one core: x [s_tiles*128, DIN] -> y."""
    _ensure_path()
    from contextlib import ExitStack

    import concourse.bacc as bacc
    import concourse.tile as tile
    from concourse import mybir
    from concourse.masks import make_identity

    f32 = mybir.dt.float32
    bf16 = mybir.dt.bfloat16
    Alu = mybir.AluOpType
    X = mybir.AxisListType.X
    Copy = mybir.ActivationFunctionType.Copy
    SROWS = s_tiles * 128

    nc = bacc.Bacc("TRN2", target_bir_lowering=False, debug=False, num_devices=N_CORES)
    x_d = nc.dram_tensor("x", [SROWS, DIN], f32, kind="ExternalInput").ap()
    w_d = nc.dram_tensor("w", [DOUT, DIN], f32, kind="ExternalInput").ap()
    b_d = nc.dram_tensor("bias", [1, DOUT], f32, kind="ExternalInput").ap()
    y_d = nc.dram_tensor("y", [SROWS, DOUT], bf16, kind="ExternalOutput").ap()

    KC = DIN // 128  # 4 contraction chunks
    OC = DOUT // 128  # 16 output chunks

    with tile.TileContext(nc) as tc, ExitStack() as ctx:
        cpool = ctx.enter_context(tc.tile_pool(name="const", bufs=1))
        wallp = ctx.enter_context(tc.tile_pool(name="wall", bufs=1))
        wtmpp = ctx.enter_context(tc.tile_pool(name="wtmp", bufs=3))
        wqp = ctx.enter_context(tc.tile_pool(name="wq", bufs=3))
        statp = ctx.enter_context(tc.tile_pool(name="stat", bufs=1))
        tTp = ctx.enter_context(tc.tile_pool(name="tT", bufs=1))
        xp = ctx.enter_context(tc.tile_pool(name="x", bufs=8))
        r1p = ctx.enter_context(tc.tile_pool(name="r1", bufs=3))
        qp = ctx.enter_context(tc.tile_pool(name="q", bufs=3))
        qtp = ctx.enter_context(tc.tile_pool(name="qt", bufs=4))
        mxp = ctx.enter_context(tc.tile_pool(name="mx", bufs=16))
        yfp = ctx.enter_context(tc.tile_pool(name="yf", bufs=3))
        yp = ctx.enter_context(tc.tile_pool(name="y", bufs=3))
        pt = ctx.enter_context(tc.tile_pool(name="ptrans", bufs=2, space="PSUM"))
        py = ctx.enter_context(tc.tile_pool(name="py", bufs=3, space="PSUM"))

        # ---- W load first: per-chunk DMAs so abs-sums start immediately, and
        # nothing else sits ahead of them in the sync HWDGE FIFO ----
        w_all = wallp.tile([128, OC, DIN], f32)
        w_r = w_d.rearrange("(c p) d -> p c d", p=128)
        for c in range(OC):
            nc.sync.dma_start(w_all[:, c : c + 1, :], w_r[:, c : c + 1, :])

        # ---- constants ----
        b_row = cpool.tile([1, DOUT], f32)
        nc.scalar.dma_start(b_row[:], b_d)
        bias_bc = cpool.tile([128, DOUT], f32)
        nc.gpsimd.dma_start(bias_bc[:], b_d.broadcast_to([128, DOUT]))
        ones128 = cpool.tile([128, 128], f32)
        nc.vector.memset(ones128[:], 1.0)
        ident = cpool.tile([128, 128], bf16)
        make_identity(nc, ident[:])

        # ---- mean(|W|): exact-split summation ----
        wsum = statp.tile([128, OC], f32)
        for c in range(OC):
            nc.vector.tensor_reduce(
                wsum[:, c : c + 1], w_all[:, c, :],
                axis=X, op=Alu.add, apply_absolute_value=True,
            )
        # split per-chunk sums (<= ~26) to a 2^-11 grid -> exact 16-way add
        hh = statp.tile([128, OC], f32)
        ll = statp.tile([128, OC], f32)
        nc.vector.tensor_scalar(hh[:], wsum[:], C_GRID_11, C_GRID_11, op0=Alu.add, op1=Alu.subtract)
        nc.vector.tensor_tensor(ll[:], wsum[:], hh[:], op=Alu.subtract)
        hs = statp.tile([128, 1], f32)
        ls = statp.tile([128, 1], f32)
        nc.vector.tensor_reduce(hs[:], hh[:], axis=X, op=Alu.add)
        nc.vector.tensor_reduce(ls[:], ll[:], axis=X, op=Alu.add)
        # split per-partition totals (<= ~400) to a 2^-4 grid -> exact 128-way add
        red = statp.tile([128, 2], f32)
        l2 = statp.tile([128, 1], f32)
        nc.vector.tensor_scalar(red[:, 0:1], hs[:], C_GRID_4, C_GRID_4, op0=Alu.add, op1=Alu.subtract)
        nc.vector.tensor_tensor(l2[:], hs[:], red[:, 0:1], op=Alu.subtract)
        nc.vector.tensor_tensor(red[:, 1:2], l2[:], ls[:], op=Alu.add)
        # cross-partition sum + broadcast in one exact fp32 ones-matmul:
        # out[m, j] = sum_p red[p, j] for every m
        pred = py.tile([128, 1024], f32, tag="ytile", name="pred")
        nc.tensor.matmul(pred[:, 0:2], ones128[:], red[:], start=True, stop=True)
        redo = statp.tile([128, 2], f32)
        nc.scalar.copy(redo[:], pred[:, 0:2])
        ssum = statp.tile([128, 1], f32)
        nc.vector.tensor_tensor(ssum[:], redo[:, 0:1], redo[:, 1:2], op=Alu.add)
        mean_t = statp.tile([128, 1], f32)
        nc.vector.tensor_scalar(mean_t[:], ssum[:], 1.0 / (DOUT * DIN), None, op0=Alu.mult)
        nc.vector.tensor_scalar(mean_t[:], mean_t[:], EPS, None, op0=Alu.max)
        s_w = statp.tile([128, 1], f32)  # 1/mean: the quantization scale
        nc.vector.reciprocal(s_w[:], mean_t[:])
        v_w = statp.tile([128, 1], f32)  # fl(1/s_w): the dequant magnitude (matches ref)
        nc.vector.reciprocal(v_w[:], s_w[:])
        vw127 = statp.tile([128, 1], f32)  # v_w / 127, folded once into the qs scale
        nc.vector.tensor_scalar(vw127[:], v_w[:], 1.0 / 127.0, None, op0=Alu.mult)

        # ---- W quantize + PE-transpose: tT[:, k, o] = t[o, k*128 + p] ----
        tT = tTp.tile([128, KC, DOUT], bf16)
        for c in range(OC):
            wc = w_all[:, c, :]
            wr1 = wtmpp.tile([128, DIN], f32, tag="wr1")
            nc.scalar.activation(wr1[:], wc, Copy, bias=MAGIC, scale=s_w[:])
            wr2 = wtmpp.tile([128, DIN], f32, tag="wr2")
            nc.vector.tensor_scalar(wr2[:], wr1[:], MAGIC, 1.0, op0=Alu.subtract, op1=Alu.min)
            wq = wqp.tile([128, DIN], bf16)
            nc.vector.tensor_scalar(wq[:], wr2[:], -1.0, None, op0=Alu.max)
            ptw = pt.tile([128, 512], bf16, tag="tp", name=f"ptw{c}")
            for k in range(KC):
                nc.tensor.transpose(
                    ptw[:, k * 128 : (k + 1) * 128], wq[:, k * 128 : (k + 1) * 128], ident[:]
                )
            # ptw[p, k*128+m] = t[c*128+m, k*128+p] -> tT[:, k, c*128+m]
            nc.vector.tensor_copy(
                tT[:, :, c * 128 : (c + 1) * 128],
                ptw[:].rearrange("p (k m) -> p k m", k=KC),
            )

        # ---- main loop over token tiles (epilogue software-pipelined by 1) ----
        state = {}

        def epilogue(j, ph0, ph1):
            ysb = yp.tile([128, DOUT], bf16)
            # half A: DVE adds bias straight out of PSUM, downcasts to bf16
            nc.vector.tensor_tensor(ysb[:, 0:1024], ph0[:], bias_bc[:, 0:1024], op=Alu.add)
            # half B: ACT stages PSUM->SBUF, GpSimd adds bias (it can't see PSUM)
            yf = yfp.tile([128, 1024], f32)
            nc.scalar.copy(yf[:], ph1[:])
            nc.gpsimd.tensor_tensor(ysb[:, 1024:2048], yf[:], bias_bc[:, 1024:2048], op=Alu.add)
            nc.gpsimd.dma_start(y_d[j * 128 : (j + 1) * 128, :], ysb[:])

        for i in range(s_tiles):
            xt = xp.tile([128, DIN], f32)
            nc.sync.dma_start(xt[:], x_d[i * 128 : (i + 1) * 128, :])

            mx = mxp.tile([128, 1], f32, tag="mx")
            nc.vector.tensor_reduce(mx[:], xt[:], axis=X, op=Alu.max, apply_absolute_value=True)
            sx = mxp.tile([128, 1], f32, tag="sx")
            nc.vector.reciprocal(sx[:], mx[:])
            nc.vector.tensor_scalar(sx[:], sx[:], 127.0, None, op0=Alu.mult)
            c_tok = mxp.tile([128, 1], f32, tag="ct")
            nc.vector.tensor_tensor(c_tok[:], mx[:], vw127[:], op=Alu.mult)

            r1 = r1p.tile([128, DIN], f32)
            nc.scalar.activation(r1[:], xt[:], Copy, bias=MAGIC, scale=sx[:])
            # qs = (round(x*sx)) * c_tok, rounded once to bf16
            qs = qp.tile([128, DIN], bf16)
            nc.vector.tensor_scalar(qs[:], r1[:], MAGIC, c_tok[:], op0=Alu.subtract, op1=Alu.mult)

            pq = pt.tile([128, 512], bf16, tag="tp", name="pq")
            for k in range(KC):
                nc.tensor.transpose(
                    pq[:, k * 128 : (k + 1) * 128], qs[:, k * 128 : (k + 1) * 128], ident[:]
                )
            qT = qtp.tile([128, KC, 128], bf16)
            nc.vector.tensor_copy(qT[:], pq[:].rearrange("p (k m) -> p k m", k=KC))

            phs = []
            for h in range(2):
                ph = py.tile([128, 1024], f32, tag="ytile", name="ph")
                base = h * 1024
                for k in range(KC):
                    lhsT = qT[:, k, :]
                    for n in range(2):
                        nc.tensor.matmul(
                            ph[:, n * 512 : (n + 1) * 512], lhsT,
                            tT[:, k, base + n * 512 : base + (n + 1) * 512],
                            start=(k == 0), stop=(k == KC - 1),
                        )
                phs.append(ph)

            if i > 0:
                epilogue(i - 1, *state["phs"])
            state = {"phs": phs}

        epilogue(s_tiles - 1, *state["phs"])

    nc.compile()
    return nc


def _get_program():
    if "nc" not in _cached:
        _cached["nc"] = build_program()
    return _cached["nc"]


def kernel(x: np.ndarray, weight: np.ndarray, bias: np.ndarray) -> np.ndarray:
    _ensure_path()
    from concourse.bass_utils import run_bass_kernel_spmd

    x = np.ascontiguousarray(x, dtype=np.float32)
    weight = np.ascontiguousarray(weight, dtype=np.float32)
    bias2d = np.ascontiguousarray(bias, dtype=np.float32).reshape(1, DOUT)

    nc = _get_program()
    in_maps = [
        {"x": x[c], "w": weight, "bias": bias2d} for c in range(N_CORES)
    ]
    res = run_bass_kernel_spmd(nc, in_maps, core_ids=list(range(N_CORES)))
    _cached["last_results"] = res
    y = np.stack(
        [np.asarray(res.results[c]["y"]).astype(np.float32) for c in range(N_CORES)],
        axis=0,
    )
    return y


# revision 13
# speedup vs baseline: 1.6163x; 1.0836x over previous
"""BitLinear (BitNet b1.58) forward kernel for Trainium2, 8 NeuronCores.

y = act_quant(x) @ weight_quant(W)^T + bias
  - activation quant: per-token absmax int8 fake-quant (values in [-127,127])
  - weight quant: per-tensor mean-absmax ternary fake-quant {-1,0,1}

Sharding: data-parallel over the batch dim (8 batches -> 1 per core);
W and bias are replicated per core, each core computes mean(|W|) locally
(no collectives needed).

Numerics (rel err ~3.4e-3 vs fp32 reference, gate is 2e-2):
  * q = round(x * s) with s = 127*recip(mx) gives exact int8 values; they are
    pre-scaled by c_tok = mx*mean|W|/127 and rounded once to bf16 (qs), so
    PSUM accumulates y - bias directly and the epilogue is a pure
    bias-add + bf16 downcast. The ternary t in {-1,0,1} is exact in bf16.
  * mean(|W|) must match the reference's fp32 value to ~2e-7 relative (nearest
    weight sits 2.6e-7 from a ternary rounding boundary), so the reduction
    uses an exact hi/lo split summation.
  * y is stored bf16 and upcast on the host.

Engine layout per token tile (steady state, software-pipelined by 1 tile):
  PE     : 16 N=512 bf16 matmuls + 4 128x128 transposes of qs. (The DMA-xbar
           transpose path is poison: the Tile scheduler serializes it against
           all in-flight DMA queues. PE transposes are nearly free.)
  ACT    : r1 = x*sx+MAGIC; qT copy PSUM->SBUF; epilogue stage ph1 PSUM->SBUF
  DVE    : absmax, recip smalls, qs = (r1-MAGIC)*c_tok -> bf16, and BOTH
           epilogue bias-adds (+bf16 downcast)
  GpSimd : IDLE. GpSimd bulk ops grab the SBUF port pair shared with DVE and
           fully block DVE's 2-port ops; SWDGE triggers starve the same port.
  sync Q : x-load triggers + y-store triggers (HWDGE)

Startup: tile 0's matmul n-groups are interleaved with the W-quant chunk
groups so the PE starts ~9us earlier (group g only needs W chunks 4g..4g+3).
"""

import os
import sys

import numpy as np

B, S, DIN, DOUT = 8, 4096, 512, 2048
N_CORES = 8

MAGIC = 12582912.0  # 1.5 * 2^23: (v + MAGIC) - MAGIC == round-half-even(v), |v| < 2^22
C_GRID_11 = 6144.0  # 1.5 * 2^12: rounds to multiples of 2^-11 (values <= ~26)
C_GRID_4 = 786432.0  # 1.5 * 2^19: rounds to multiples of 2^-4  (values <= ~400)
EPS = 1e-6

_cached = {}


def _ensure_path():
    try:
        import concourse  # noqa: F401
    except ImportError:
        for p in ("/opt/trn_rl_repo", os.path.expanduser("~/.axon_site/_ro/trn_rl_repo")):
            if os.path.isdir(p) and p not in sys.path:
                sys.path.insert(0, p)


def build_program(s_tiles=S // 128):
    """Emit the Bass/Tile program for one core: x [s_tiles*128, DIN] -> y."""
    _ensure_path()
    from contextlib import ExitStack

    import concourse.bacc as bacc
    import concourse.tile as tile
    from concourse import mybir
    from concourse.masks import make_identity

    f32 = mybir.dt.float32
    bf16 = mybir.dt.bfloat16
    Alu = mybir.AluOpType
    X = mybir.AxisListType.X
    Copy = mybir.ActivationFunctionType.Copy
    SROWS = s_tiles * 128

    nc = bacc.Bacc("TRN2", target_bir_lowering=False, debug=False, num_devices=N_CORES)
    x_d = nc.dram_tensor("x", [SROWS, DIN], f32, kind="ExternalInput").ap()
    w_d = nc.dram_tensor("w", [DOUT, DIN], f32, kind="ExternalInput").ap()
    b_d = nc.dram_tensor("bias", [1, DOUT], f32, kind="ExternalInput").ap()
    y_d = nc.dram_tensor("y", [SROWS, DOUT], bf16, kind="ExternalOutput").ap()

    KC = DIN // 128  # 4 contraction chunks
    OC = DOUT // 128  # 16 output chunks

    with tile.TileContext(nc) as tc, ExitStack() as ctx:
        cpool = ctx.enter_context(tc.tile_pool(name="const", bufs=1))
        wallp = ctx.enter_context(tc.tile_pool(name="wall", bufs=1))
        wtmpp = ctx.enter_context(tc.tile_pool(name="wtmp", bufs=3))
        wqp = ctx.enter_context(tc.tile_pool(name="wq", bufs=3))
        statp = ctx.enter_context(tc.tile_pool(name="stat", bufs=1))
        tTp = ctx.enter_context(tc.tile_pool(name="tT", bufs=1))
        xp = ctx.enter_context(tc.tile_pool(name="x", bufs=8))
        r1p = ctx.enter_context(tc.tile_pool(name="r1", bufs=4))
        qp = ctx.enter_context(tc.tile_pool(name="q", bufs=4))
        qtp = ctx.enter_context(tc.tile_pool(name="qt", bufs=6))
        mxp = ctx.enter_context(tc.tile_pool(name="mx", bufs=16))
        yfp = ctx.enter_context(tc.tile_pool(name="yf", bufs=4))
        yp = ctx.enter_context(tc.tile_pool(name="y", bufs=5))
        pt = ctx.enter_context(tc.tile_pool(name="ptrans", bufs=2, space="PSUM"))
        py = ctx.enter_context(tc.tile_pool(name="py", bufs=3, space="PSUM"))

        # ---- W load first: per-chunk DMAs so abs-sums start immediately, and
        # nothing else sits ahead of them in the sync HWDGE FIFO ----
        w_all = wallp.tile([128, OC, DIN], f32)
        w_r = w_d.rearrange("(c p) d -> p c d", p=128)
        for c in range(OC):
            nc.sync.dma_start(w_all[:, c : c + 1, :], w_r[:, c : c + 1, :])

        # ---- constants ----
        b_row = cpool.tile([1, DOUT], f32)
        nc.scalar.dma_start(b_row[:], b_d)
        bias_bc = cpool.tile([128, DOUT], f32)
        nc.gpsimd.dma_start(bias_bc[:], b_d.broadcast_to([128, DOUT]))
        ones128 = cpool.tile([128, 128], f32)
        nc.vector.memset(ones128[:], 1.0)
        ident = cpool.tile([128, 128], bf16)
        make_identity(nc, ident[:])

        # ---- mean(|W|): exact-split summation ----
        wsum = statp.tile([128, OC], f32)
        for c in range(OC):
            nc.vector.tensor_reduce(
                wsum[:, c : c + 1], w_all[:, c, :],
                axis=X, op=Alu.add, apply_absolute_value=True,
            )
        # split per-chunk sums (<= ~26) to a 2^-11 grid -> exact 16-way add
        hh = statp.tile([128, OC], f32)
        ll = statp.tile([128, OC], f32)
        nc.vector.tensor_scalar(hh[:], wsum[:], C_GRID_11, C_GRID_11, op0=Alu.add, op1=Alu.subtract)
        nc.vector.tensor_tensor(ll[:], wsum[:], hh[:], op=Alu.subtract)
        hs = statp.tile([128, 1], f32)
        ls = statp.tile([128, 1], f32)
        nc.vector.tensor_reduce(hs[:], hh[:], axis=X, op=Alu.add)
        nc.vector.tensor_reduce(ls[:], ll[:], axis=X, op=Alu.add)
        # split per-partition totals (<= ~400) to a 2^-4 grid -> exact 128-way add
        red = statp.tile([128, 2], f32)
        l2 = statp.tile([128, 1], f32)
        nc.vector.tensor_scalar(red[:, 0:1], hs[:], C_GRID_4, C_GRID_4, op0=Alu.add, op1=Alu.subtract)
        nc.vector.tensor_tensor(l2[:], hs[:], red[:, 0:1], op=Alu.subtract)
        nc.vector.tensor_tensor(red[:, 1:2], l2[:], ls[:], op=Alu.add)
        # cross-partition sum + broadcast in one exact fp32 ones-matmul:
        # out[m, j] = sum_p red[p, j] for every m
        pred = py.tile([128, 1024], f32, tag="ytile", name="pred")
        nc.tensor.matmul(pred[:, 0:2], ones128[:], red[:], start=True, stop=True)
        redo = statp.tile([128, 2], f32)
        nc.scalar.copy(redo[:], pred[:, 0:2])
        ssum = statp.tile([128, 1], f32)
        nc.vector.tensor_tensor(ssum[:], redo[:, 0:1], redo[:, 1:2], op=Alu.add)
        mean_t = statp.tile([128, 1], f32)
        nc.vector.tensor_scalar(mean_t[:], ssum[:], 1.0 / (DOUT * DIN), None, op0=Alu.mult)
        nc.vector.tensor_scalar(mean_t[:], mean_t[:], EPS, None, op0=Alu.max)
        s_w = statp.tile([128, 1], f32)  # 1/mean: the quantization scale
        nc.vector.reciprocal(s_w[:], mean_t[:])
        v_w = statp.tile([128, 1], f32)  # fl(1/s_w): the dequant magnitude (matches ref)
        nc.vector.reciprocal(v_w[:], s_w[:])
        vw127 = statp.tile([128, 1], f32)  # v_w / 127, folded once into the qs scale
        nc.vector.tensor_scalar(vw127[:], v_w[:], 1.0 / 127.0, None, op0=Alu.mult)

        # ---- W quantize + PE-transpose, emitted in 4-chunk groups so tile 0's
        # matmul group g can start as soon as W chunks 4g..4g+3 are ready ----
        tT = tTp.tile([128, KC, DOUT], bf16)

        def quant_w_chunk(c):
            wc = w_all[:, c, :]
            wr1 = wtmpp.tile([128, DIN], f32, tag="wr1")
            nc.scalar.activation(wr1[:], wc, Copy, bias=MAGIC, scale=s_w[:])
            wr2 = wtmpp.tile([128, DIN], f32, tag="wr2")
            nc.vector.tensor_scalar(wr2[:], wr1[:], MAGIC, 1.0, op0=Alu.subtract, op1=Alu.min)
            wq = wqp.tile([128, DIN], bf16)
            nc.vector.tensor_scalar(wq[:], wr2[:], -1.0, None, op0=Alu.max)
            ptw = pt.tile([128, 512], bf16, tag="tp", name=f"ptw{c}")
            for k in range(KC):
                nc.tensor.transpose(
                    ptw[:, k * 128 : (k + 1) * 128], wq[:, k * 128 : (k + 1) * 128], ident[:]
                )
            # ptw[p, k*128+m] = t[c*128+m, k*128+p] -> tT[:, k, c*128+m]
            dst = tT[:, :, c * 128 : (c + 1) * 128]
            src = ptw[:].rearrange("p (k m) -> p k m", k=KC)
            if c % 2 == 0:
                nc.vector.tensor_copy(dst, src)
            else:
                nc.scalar.copy(dst, src)

        # ---- main-loop building blocks (software-pipelined by 1 tile) ----
        state = {}
        xts = {}

        def load_x(i):
            xt = xp.tile([128, DIN], f32)
            nc.sync.dma_start(xt[:], x_d[i * 128 : (i + 1) * 128, :])
            return xt

        def frontend(i):
            xt = xts.pop(i) if i in xts else load_x(i)
            mx = mxp.tile([128, 1], f32, tag="mx")
            nc.vector.tensor_reduce(mx[:], xt[:], axis=X, op=Alu.max, apply_absolute_value=True)
            sx = mxp.tile([128, 1], f32, tag="sx")
            nc.vector.reciprocal(sx[:], mx[:])
            nc.vector.tensor_scalar(sx[:], sx[:], 127.0, None, op0=Alu.mult)
            c_tok = mxp.tile([128, 1], f32, tag="ct")
            nc.vector.tensor_tensor(c_tok[:], mx[:], vw127[:], op=Alu.mult)
            r1 = r1p.tile([128, DIN], f32)
            nc.scalar.activation(r1[:], xt[:], Copy, bias=MAGIC, scale=sx[:])
            # qs = (round(x*sx)) * c_tok, rounded once to bf16
            qs = qp.tile([128, DIN], bf16)
            nc.vector.tensor_scalar(qs[:], r1[:], MAGIC, c_tok[:], op0=Alu.subtract, op1=Alu.mult)
            return qs

        def transposes(qs):
            pq = pt.tile([128, 512], bf16, tag="tp", name="pq")
            for k in range(KC):
                nc.tensor.transpose(
                    pq[:, k * 128 : (k + 1) * 128], qs[:, k * 128 : (k + 1) * 128], ident[:]
                )
            qT = qtp.tile([128, KC, 128], bf16)
            nc.scalar.copy(qT[:], pq[:].rearrange("p (k m) -> p k m", k=KC))
            return qT

        def alloc_ph():
            return [py.tile([128, 1024], f32, tag="ytile", name="ph") for _ in range(2)]

        def mm_group(phs, qT, g):
            ph = phs[g // 2]
            nsl = (g % 2) * 512
            for k in range(KC):
                nc.tensor.matmul(
                    ph[:, nsl : nsl + 512], qT[:, k, :],
                    tT[:, k, g * 512 : (g + 1) * 512],
                    start=(k == 0), stop=(k == KC - 1),
                )

        def epilogue(j, phs):
            ph0, ph1 = phs
            ysb = yp.tile([128, DOUT], bf16)
            # half A: DVE adds bias straight out of PSUM, downcasts to bf16
            nc.vector.tensor_tensor(ysb[:, 0:1024], ph0[:], bias_bc[:, 0:1024], op=Alu.add)
            # half B: ACT stages PSUM->SBUF, DVE adds bias from SBUF
            yf = yfp.tile([128, 1024], f32)
            nc.scalar.copy(yf[:], ph1[:])
            nc.vector.tensor_tensor(ysb[:, 1024:2048], yf[:], bias_bc[:, 1024:2048], op=Alu.add)
            nc.sync.dma_start(y_d[j * 128 : (j + 1) * 128, :], ysb[:])

        # ---- startup: interleave tile 0 with the W-quant groups ----
        for i in range(4):
            xts[i] = load_x(i)  # prefetch behind the W chunks in the DMA queue
        for c in range(4):
            quant_w_chunk(c)
        qs0 = frontend(0)
        qT0 = transposes(qs0)
        phs0 = alloc_ph()
        mm_group(phs0, qT0, 0)
        for g in range(1, 4):
            for c in range(4 * g, 4 * g + 4):
                quant_w_chunk(c)
            mm_group(phs0, qT0, g)
        state = {"phs": phs0}

        # ---- steady loop ----
        for i in range(1, s_tiles):
            qs = frontend(i)
            qT = transposes(qs)
            phs = alloc_ph()
            for g in range(4):
                mm_group(phs, qT, g)
            epilogue(i - 1, state["phs"])
            state = {"phs": phs}

        epilogue(s_tiles - 1, state["phs"])

    nc.compile()
    return nc


def _get_program():
    if "nc" not in _cached:
        _cached["nc"] = build_program()
    return _cached["nc"]


def kernel(x: np.ndarray, weight: np.ndarray, bias: np.ndarray) -> np.ndarray:
    _ensure_path()
    from concourse.bass_utils import run_bass_kernel_spmd

    x = np.ascontiguousarray(x, dtype=np.float32)
    weight = np.ascontiguousarray(weight, dtype=np.float32)
    bias2d = np.ascontiguousarray(bias, dtype=np.float32).reshape(1, DOUT)

    nc = _get_program()
    in_maps = [
        {"x": x[c], "w": weight, "bias": bias2d} for c in range(N_CORES)
    ]
    res = run_bass_kernel_spmd(nc, in_maps, core_ids=list(range(N_CORES)))
    _cached["last_results"] = res
    y = np.stack(
        [np.asarray(res.results[c]["y"]).astype(np.float32) for c in range(N_CORES)],
        axis=0,
    )
    return y


# revision 15
# speedup vs baseline: 1.8067x; 1.1178x over previous
"""BitLinear (BitNet b1.58) forward kernel for Trainium2, 8 NeuronCores.

y = act_quant(x) @ weight_quant(W)^T + bias
  - activation quant: per-token absmax int8 fake-quant (values in [-127,127])
  - weight quant: per-tensor mean-absmax ternary fake-quant {-1,0,1}

Sharding: data-parallel over the batch dim (8 batches -> 1 per core);
W and bias are replicated per core, each core computes mean(|W|) locally
(no collectives needed).

Numerics (rel err ~3.4e-3 vs fp32 reference, gate is 2e-2):
  * q = round(x * s) with s = 127*recip(mx) gives exact int8 values; they are
    pre-scaled by c_tok = mx*mean|W|/127 and rounded once to bf16 (qs), so
    PSUM accumulates y - bias directly and the epilogue is a pure
    bias-add + bf16 downcast. The ternary t in {-1,0,1} is exact in bf16.
  * mean(|W|) must match the reference's fp32 value to ~2e-7 relative (nearest
    weight sits 2.6e-7 from a ternary rounding boundary), so the reduction
    uses an exact hi/lo split summation.
  * y is stored bf16 and upcast on the host.

Engine layout per token tile (steady state):
  PE     : 4 128x128 transposes of qs (one tile ahead) + 16 N=512 bf16
           matmuls. (The DMA-xbar transpose path is poison: the Tile
           scheduler serializes it against all in-flight DMA queues.)
  ACT    : r1 = x*sx+MAGIC; qT copy PSUM->SBUF (hidden under the MM stream)
  DVE    : absmax, recip smalls, qs = (r1-MAGIC)*c_tok -> bf16, and the whole
           epilogue: both [128,1024] halves PSUM + bias -> bf16
  GpSimd : IDLE (GpSimd bulk ops grab the SBUF port pair shared with DVE and
           fully block DVE's 2-port ops; SWDGE triggers starve the same port)
  sync Q : x-load + y-store triggers (HWDGE; each trigger costs ~0.6us of
           queue time, so W loads are batched 2 chunks/DMA over sync+scalar)

Pipelining: front-end runs 2 tiles ahead, transposes 1 tile ahead, epilogue
1 tile behind; tile 0's four matmul n-groups interleave with the W-quant
chunk groups (group g only needs W chunks 4g..4g+3).
"""

import os
import sys

import numpy as np

B, S, DIN, DOUT = 8, 4096, 512, 2048
N_CORES = 8

MAGIC = 12582912.0  # 1.5 * 2^23: (v + MAGIC) - MAGIC == round-half-even(v), |v| < 2^22
C_GRID_11 = 6144.0  # 1.5 * 2^12: rounds to multiples of 2^-11 (values <= ~26)
C_GRID_4 = 786432.0  # 1.5 * 2^19: rounds to multiples of 2^-4  (values <= ~400)
EPS = 1e-6

_cached = {}


def _ensure_path():
    try:
        import concourse  # noqa: F401
    except ImportError:
        for p in ("/opt/trn_rl_repo", os.path.expanduser("~/.axon_site/_ro/trn_rl_repo")):
            if os.path.isdir(p) and p not in sys.path:
                sys.path.insert(0, p)


def build_program(s_tiles=S // 128):
    """Emit the Bass/Tile program for one core: x [s_tiles*128, DIN] -> y."""
    _ensure_path()
    from contextlib import ExitStack

    import concourse.bacc as bacc
    import concourse.tile as tile
    from concourse import mybir
    from concourse.masks import make_identity

    f32 = mybir.dt.float32
    bf16 = mybir.dt.bfloat16
    Alu = mybir.AluOpType
    X = mybir.AxisListType.X
    Copy = mybir.ActivationFunctionType.Copy
    SROWS = s_tiles * 128

    nc = bacc.Bacc("TRN2", target_bir_lowering=False, debug=False, num_devices=N_CORES)
    x_d = nc.dram_tensor("x", [SROWS, DIN], f32, kind="ExternalInput").ap()
    w_d = nc.dram_tensor("w", [DOUT, DIN], f32, kind="ExternalInput").ap()
    b_d = nc.dram_tensor("bias", [1, DOUT], f32, kind="ExternalInput").ap()
    y_d = nc.dram_tensor("y", [SROWS, DOUT], bf16, kind="ExternalOutput").ap()

    KC = DIN // 128  # 4 contraction chunks
    OC = DOUT // 128  # 16 output chunks

    with tile.TileContext(nc) as tc, ExitStack() as ctx:
        cpool = ctx.enter_context(tc.tile_pool(name="const", bufs=1))
        wallp = ctx.enter_context(tc.tile_pool(name="wall", bufs=1))
        wtmpp = ctx.enter_context(tc.tile_pool(name="wtmp", bufs=3))
        wqp = ctx.enter_context(tc.tile_pool(name="wq", bufs=3))
        statp = ctx.enter_context(tc.tile_pool(name="stat", bufs=1))
        tTp = ctx.enter_context(tc.tile_pool(name="tT", bufs=1))
        xp = ctx.enter_context(tc.tile_pool(name="x", bufs=8))
        r1p = ctx.enter_context(tc.tile_pool(name="r1", bufs=4))
        qp = ctx.enter_context(tc.tile_pool(name="q", bufs=4))
        qtp = ctx.enter_context(tc.tile_pool(name="qt", bufs=6))
        mxp = ctx.enter_context(tc.tile_pool(name="mx", bufs=24))
        yp = ctx.enter_context(tc.tile_pool(name="y", bufs=5))
        pt = ctx.enter_context(tc.tile_pool(name="ptrans", bufs=2, space="PSUM"))
        py = ctx.enter_context(tc.tile_pool(name="py", bufs=3, space="PSUM"))

        # ---- W load first: 2 chunks per DMA, triggers split across the two
        # HWDGE queues (each trigger costs ~0.6us of queue sequencer time) ----
        w_all = wallp.tile([128, OC, DIN], f32)
        w_r = w_d.rearrange("(c p) d -> p c d", p=128)
        for g in range(OC // 2):
            q_eng = nc.sync if g % 2 == 0 else nc.scalar
            q_eng.dma_start(w_all[:, 2 * g : 2 * g + 2, :], w_r[:, 2 * g : 2 * g + 2, :])

        # ---- constants ----
        bias_bc = cpool.tile([128, DOUT], f32)
        nc.gpsimd.dma_start(bias_bc[:], b_d.broadcast_to([128, DOUT]))
        ones128 = cpool.tile([128, 128], f32)
        nc.vector.memset(ones128[:], 1.0)
        ident = cpool.tile([128, 128], bf16)
        make_identity(nc, ident[:])

        # ---- x prefetch for the first tiles, behind W in the DMA queue ----
        xts = {}

        def load_x(i):
            xt = xp.tile([128, DIN], f32)
            nc.sync.dma_start(xt[:], x_d[i * 128 : (i + 1) * 128, :])
            return xt

        for i in range(4):
            xts[i] = load_x(i)

        # ---- mean(|W|): exact-split summation ----
        wsum = statp.tile([128, OC], f32)
        for c in range(OC):
            nc.vector.tensor_reduce(
                wsum[:, c : c + 1], w_all[:, c, :],
                axis=X, op=Alu.add, apply_absolute_value=True,
            )
        # split per-chunk sums (<= ~26) to a 2^-11 grid -> exact 16-way add
        hh = statp.tile([128, OC], f32)
        ll = statp.tile([128, OC], f32)
        nc.vector.tensor_scalar(hh[:], wsum[:], C_GRID_11, C_GRID_11, op0=Alu.add, op1=Alu.subtract)
        nc.vector.tensor_tensor(ll[:], wsum[:], hh[:], op=Alu.subtract)
        hs = statp.tile([128, 1], f32)
        ls = statp.tile([128, 1], f32)
        nc.vector.tensor_reduce(hs[:], hh[:], axis=X, op=Alu.add)
        nc.vector.tensor_reduce(ls[:], ll[:], axis=X, op=Alu.add)
        # split per-partition totals (<= ~400) to a 2^-4 grid -> exact 128-way add
        red = statp.tile([128, 2], f32)
        l2 = statp.tile([128, 1], f32)
        nc.vector.tensor_scalar(red[:, 0:1], hs[:], C_GRID_4, C_GRID_4, op0=Alu.add, op1=Alu.subtract)
        nc.vector.tensor_tensor(l2[:], hs[:], red[:, 0:1], op=Alu.subtract)
        nc.vector.tensor_tensor(red[:, 1:2], l2[:], ls[:], op=Alu.add)
        # cross-partition sum + broadcast in one exact fp32 ones-matmul:
        # out[m, j] = sum_p red[p, j] for every m
        pred = py.tile([128, 1024], f32, tag="ytile", name="pred")
        nc.tensor.matmul(pred[:, 0:2], ones128[:], red[:], start=True, stop=True)
        redo = statp.tile([128, 2], f32)
        nc.scalar.copy(redo[:], pred[:, 0:2])
        ssum = statp.tile([128, 1], f32)
        nc.vector.tensor_tensor(ssum[:], redo[:, 0:1], redo[:, 1:2], op=Alu.add)
        mean_t = statp.tile([128, 1], f32)
        nc.vector.tensor_scalar(mean_t[:], ssum[:], 1.0 / (DOUT * DIN), None, op0=Alu.mult)
        nc.vector.tensor_scalar(mean_t[:], mean_t[:], EPS, None, op0=Alu.max)
        s_w = statp.tile([128, 1], f32)  # 1/mean: the quantization scale
        nc.vector.reciprocal(s_w[:], mean_t[:])
        v_w = statp.tile([128, 1], f32)  # fl(1/s_w): the dequant magnitude (matches ref)
        nc.vector.reciprocal(v_w[:], s_w[:])
        vw127 = statp.tile([128, 1], f32)  # v_w / 127, folded once into the qs scale
        nc.vector.tensor_scalar(vw127[:], v_w[:], 1.0 / 127.0, None, op0=Alu.mult)

        # ---- W quantize + PE-transpose, in 4-chunk groups ----
        tT = tTp.tile([128, KC, DOUT], bf16)

        def quant_w_chunk(c):
            wc = w_all[:, c, :]
            wr1 = wtmpp.tile([128, DIN], f32, tag="wr1")
            nc.scalar.activation(wr1[:], wc, Copy, bias=MAGIC, scale=s_w[:])
            wr2 = wtmpp.tile([128, DIN], f32, tag="wr2")
            nc.vector.tensor_scalar(wr2[:], wr1[:], MAGIC, 1.0, op0=Alu.subtract, op1=Alu.min)
            wq = wqp.tile([128, DIN], bf16)
            nc.vector.tensor_scalar(wq[:], wr2[:], -1.0, None, op0=Alu.max)
            ptw = pt.tile([128, 512], bf16, tag="tp", name=f"ptw{c}")
            for k in range(KC):
                nc.tensor.transpose(
                    ptw[:, k * 128 : (k + 1) * 128], wq[:, k * 128 : (k + 1) * 128], ident[:]
                )
            # ptw[p, k*128+m] = t[c*128+m, k*128+p] -> tT[:, k, c*128+m]
            dst = tT[:, :, c * 128 : (c + 1) * 128]
            src = ptw[:].rearrange("p (k m) -> p k m", k=KC)
            if c % 2 == 0:
                nc.vector.tensor_copy(dst, src)
            else:
                nc.scalar.copy(dst, src)

        # ---- per-tile building blocks ----
        def frontend(i):
            xt = xts.pop(i) if i in xts else load_x(i)
            mx = mxp.tile([128, 1], f32, tag="mx")
            nc.vector.tensor_reduce(mx[:], xt[:], axis=X, op=Alu.max, apply_absolute_value=True)
            sx = mxp.tile([128, 1], f32, tag="sx")
            nc.vector.reciprocal(sx[:], mx[:])
            nc.vector.tensor_scalar(sx[:], sx[:], 127.0, None, op0=Alu.mult)
            c_tok = mxp.tile([128, 1], f32, tag="ct")
            nc.vector.tensor_tensor(c_tok[:], mx[:], vw127[:], op=Alu.mult)
            r1 = r1p.tile([128, DIN], f32)
            nc.scalar.activation(r1[:], xt[:], Copy, bias=MAGIC, scale=sx[:])
            # qs = (round(x*sx)) * c_tok, rounded once to bf16
            qs = qp.tile([128, DIN], bf16)
            nc.vector.tensor_scalar(qs[:], r1[:], MAGIC, c_tok[:], op0=Alu.subtract, op1=Alu.mult)
            return qs

        def transposes(qs):
            pq = pt.tile([128, 512], bf16, tag="tp", name="pq")
            for k in range(KC):
                nc.tensor.transpose(
                    pq[:, k * 128 : (k + 1) * 128], qs[:, k * 128 : (k + 1) * 128], ident[:]
                )
            qT = qtp.tile([128, KC, 128], bf16)
            nc.scalar.copy(qT[:], pq[:].rearrange("p (k m) -> p k m", k=KC))
            return qT

        def alloc_ph():
            return [py.tile([128, 1024], f32, tag="ytile", name="ph") for _ in range(2)]

        def mm_group(phs, qT, g):
            ph = phs[g // 2]
            nsl = (g % 2) * 512
            for k in range(KC):
                nc.tensor.matmul(
                    ph[:, nsl : nsl + 512], qT[:, k, :],
                    tT[:, k, g * 512 : (g + 1) * 512],
                    start=(k == 0), stop=(k == KC - 1),
                )

        def epilogue(j, phs):
            ysb = yp.tile([128, DOUT], bf16)
            for h in range(2):
                nc.vector.tensor_tensor(
                    ysb[:, h * 1024 : (h + 1) * 1024], phs[h][:],
                    bias_bc[:, h * 1024 : (h + 1) * 1024], op=Alu.add,
                )
            nc.sync.dma_start(y_d[j * 128 : (j + 1) * 128, :], ysb[:])

        # ---- startup: tile 0 interleaved with the W-quant groups ----
        qs_t = {0: frontend(0)}
        for c in range(0, 4):
            quant_w_chunk(c)
        qT_t = {0: transposes(qs_t.pop(0))}
        qs_t[1] = frontend(1)
        phs0 = alloc_ph()
        mm_group(phs0, qT_t[0], 0)
        for c in range(4, 8):
            quant_w_chunk(c)
        mm_group(phs0, qT_t[0], 1)
        qT_t[1] = transposes(qs_t.pop(1))
        qs_t[2] = frontend(2)
        for c in range(8, 12):
            quant_w_chunk(c)
        mm_group(phs0, qT_t[0], 2)
        for c in range(12, 16):
            quant_w_chunk(c)
        mm_group(phs0, qT_t[0], 3)
        del qT_t[0]
        state = {"phs": phs0}

        # ---- steady loop: fe 2 ahead, transposes 1 ahead, epilogue 1 behind ----
        for i in range(1, s_tiles):
            if i + 2 <= s_tiles - 1:
                qs_t[i + 2] = frontend(i + 2)
            if i + 1 <= s_tiles - 1:
                qT_t[i + 1] = transposes(qs_t.pop(i + 1))
            phs = alloc_ph()
            qT_i = qT_t.pop(i)
            for g in range(4):
                mm_group(phs, qT_i, g)
            epilogue(i - 1, state["phs"])
            state = {"phs": phs}

        epilogue(s_tiles - 1, state["phs"])

    nc.compile()
    return nc


def _get_program():
    if "nc" not in _cached:
        _cached["nc"] = build_program()
    return _cached["nc"]


def kernel(x: np.ndarray, weight: np.ndarray, bias: np.ndarray) -> np.ndarray:
    _ensure_path()
    from concourse.bass_utils import run_bass_kernel_spmd

    x = np.ascontiguousarray(x, dtype=np.float32)
    weight = np.ascontiguousarray(weight, dtype=np.float32)
    bias2d = np.ascontiguousarray(bias, dtype=np.float32).reshape(1, DOUT)

    nc = _get_program()
    in_maps = [
        {"x": x[c], "w": weight, "bias": bias2d} for c in range(N_CORES)
    ]
    res = run_bass_kernel_spmd(nc, in_maps, core_ids=list(range(N_CORES)))
    _cached["last_results"] = res
    y = np.stack(
        [np.asarray(res.results[c]["y"]).astype(np.float32) for c in range(N_CORES)],
        axis=0,
    )
    return y


# revision 16
# speedup vs baseline: 1.8299x; 1.0129x over previous
"""BitLinear (BitNet b1.58) forward kernel for Trainium2, 8 NeuronCores.

y = act_quant(x) @ weight_quant(W)^T + bias
  - activation quant: per-token absmax int8 fake-quant (values in [-127,127])
  - weight quant: per-tensor mean-absmax ternary fake-quant {-1,0,1}

Sharding: data-parallel over the batch dim (8 batches -> 1 per core);
W and bias are replicated per core, each core computes mean(|W|) locally
(no collectives needed).

Numerics (rel err ~3.4e-3 vs fp32 reference, gate is 2e-2):
  * q = round(x * s) with s = 127*recip(mx) gives exact int8 values; they are
    pre-scaled by c_tok = mx*mean|W|/127 and rounded once to bf16 (qs), so
    PSUM accumulates y - bias directly and the epilogue is a pure
    bias-add + bf16 downcast. The ternary t in {-1,0,1} is exact in bf16.
  * mean(|W|) must match the reference's fp32 value to ~2e-7 relative (nearest
    weight sits 2.6e-7 from a ternary rounding boundary), so the reduction
    uses an exact hi/lo split summation.
  * y is stored bf16 and upcast on the host.

Engine layout per token tile (steady state):
  PE     : 4 128x128 transposes of qs (one tile ahead) + 16 N=512 bf16
           matmuls. (The DMA-xbar transpose path is poison: the Tile
           scheduler serializes it against all in-flight DMA queues.)
  ACT    : r1 = x*sx+MAGIC; qT copy PSUM->SBUF (hidden under the MM stream)
  DVE    : absmax, recip smalls, qs = (r1-MAGIC)*c_tok -> bf16, and the whole
           epilogue: both [128,1024] halves PSUM + bias -> bf16
  GpSimd : IDLE (GpSimd bulk ops grab the SBUF port pair shared with DVE and
           fully block DVE's 2-port ops; SWDGE triggers starve the same port)
  sync Q : x-load + y-store triggers (HWDGE; each trigger costs ~0.6us of
           queue time, so W loads are batched 2 chunks/DMA over sync+scalar)

Pipelining: front-end runs 2 tiles ahead, transposes 1 tile ahead, epilogue
1 tile behind; tile 0's four matmul n-groups interleave with the W-quant
chunk groups (group g only needs W chunks 4g..4g+3).
"""

import os
import sys

import numpy as np

B, S, DIN, DOUT = 8, 4096, 512, 2048
N_CORES = 8

MAGIC = 12582912.0  # 1.5 * 2^23: (v + MAGIC) - MAGIC == round-half-even(v), |v| < 2^22
C_GRID_11 = 6144.0  # 1.5 * 2^12: rounds to multiples of 2^-11 (values <= ~26)
C_GRID_4 = 786432.0  # 1.5 * 2^19: rounds to multiples of 2^-4  (values <= ~400)
EPS = 1e-6

_cached = {}


def _ensure_path():
    try:
        import concourse  # noqa: F401
    except ImportError:
        for p in ("/opt/trn_rl_repo", os.path.expanduser("~/.axon_site/_ro/trn_rl_repo")):
            if os.path.isdir(p) and p not in sys.path:
                sys.path.insert(0, p)


def build_program(s_tiles=S // 128):
    """Emit the Bass/Tile program for one core: x [s_tiles*128, DIN] -> y."""
    _ensure_path()
    from contextlib import ExitStack

    import concourse.bacc as bacc
    import concourse.tile as tile
    from concourse import mybir
    from concourse.masks import make_identity

    f32 = mybir.dt.float32
    bf16 = mybir.dt.bfloat16
    Alu = mybir.AluOpType
    X = mybir.AxisListType.X
    Copy = mybir.ActivationFunctionType.Copy
    SROWS = s_tiles * 128

    nc = bacc.Bacc("TRN2", target_bir_lowering=False, debug=False, num_devices=N_CORES)
    x_d = nc.dram_tensor("x", [SROWS, DIN], f32, kind="ExternalInput").ap()
    w_d = nc.dram_tensor("w", [DOUT, DIN], f32, kind="ExternalInput").ap()
    b_d = nc.dram_tensor("bias", [1, DOUT], f32, kind="ExternalInput").ap()
    y_d = nc.dram_tensor("y", [SROWS, DOUT], bf16, kind="ExternalOutput").ap()

    KC = DIN // 128  # 4 contraction chunks
    OC = DOUT // 128  # 16 output chunks

    with tile.TileContext(nc) as tc, ExitStack() as ctx:
        cpool = ctx.enter_context(tc.tile_pool(name="const", bufs=1))
        wallp = ctx.enter_context(tc.tile_pool(name="wall", bufs=1))
        wtmpp = ctx.enter_context(tc.tile_pool(name="wtmp", bufs=3))
        wqp = ctx.enter_context(tc.tile_pool(name="wq", bufs=3))
        statp = ctx.enter_context(tc.tile_pool(name="stat", bufs=1))
        tTp = ctx.enter_context(tc.tile_pool(name="tT", bufs=1))
        xp = ctx.enter_context(tc.tile_pool(name="x", bufs=8))
        r1p = ctx.enter_context(tc.tile_pool(name="r1", bufs=4))
        qp = ctx.enter_context(tc.tile_pool(name="q", bufs=4))
        qtp = ctx.enter_context(tc.tile_pool(name="qt", bufs=6))
        mxp = ctx.enter_context(tc.tile_pool(name="mx", bufs=24))
        yp = ctx.enter_context(tc.tile_pool(name="y", bufs=5))
        pt = ctx.enter_context(tc.tile_pool(name="ptrans", bufs=2, space="PSUM"))
        py = ctx.enter_context(tc.tile_pool(name="py", bufs=3, space="PSUM"))

        # ---- W load first: per-chunk contiguous DMAs (1 descriptor/partition),
        # triggers split across the two HWDGE queues (each trigger costs
        # ~0.6us of queue sequencer time) ----
        w_all = wallp.tile([128, OC, DIN], f32)
        w_r = w_d.rearrange("(c p) d -> p c d", p=128)
        for c in range(OC):
            q_eng = nc.sync if c % 2 == 0 else nc.scalar
            q_eng.dma_start(w_all[:, c : c + 1, :], w_r[:, c : c + 1, :])

        # ---- constants ----
        ones128 = cpool.tile([128, 128], f32)
        nc.vector.memset(ones128[:], 1.0)
        ident = cpool.tile([128, 128], bf16)
        make_identity(nc, ident[:])

        # ---- x prefetch for the first tiles, behind W in the DMA queue ----
        xts = {}

        def load_x(i):
            xt = xp.tile([128, DIN], f32)
            nc.sync.dma_start(xt[:], x_d[i * 128 : (i + 1) * 128, :])
            return xt

        for i in range(4):
            xts[i] = load_x(i)
        # bias broadcast is only needed by the first epilogue (~40us in);
        # keep its 1MB SBUF write out of the W/x load window
        bias_bc = cpool.tile([128, DOUT], f32)
        nc.gpsimd.dma_start(bias_bc[:], b_d.broadcast_to([128, DOUT]))

        # ---- mean(|W|): exact-split summation ----
        wsum = statp.tile([128, OC], f32)
        for c in range(OC):
            nc.vector.tensor_reduce(
                wsum[:, c : c + 1], w_all[:, c, :],
                axis=X, op=Alu.add, apply_absolute_value=True,
            )
        # split per-chunk sums (<= ~26) to a 2^-11 grid -> exact 16-way add
        hh = statp.tile([128, OC], f32)
        ll = statp.tile([128, OC], f32)
        nc.vector.tensor_scalar(hh[:], wsum[:], C_GRID_11, C_GRID_11, op0=Alu.add, op1=Alu.subtract)
        nc.vector.tensor_tensor(ll[:], wsum[:], hh[:], op=Alu.subtract)
        hs = statp.tile([128, 1], f32)
        ls = statp.tile([128, 1], f32)
        nc.vector.tensor_reduce(hs[:], hh[:], axis=X, op=Alu.add)
        nc.vector.tensor_reduce(ls[:], ll[:], axis=X, op=Alu.add)
        # split per-partition totals (<= ~400) to a 2^-4 grid -> exact 128-way add
        red = statp.tile([128, 2], f32)
        l2 = statp.tile([128, 1], f32)
        nc.vector.tensor_scalar(red[:, 0:1], hs[:], C_GRID_4, C_GRID_4, op0=Alu.add, op1=Alu.subtract)
        nc.vector.tensor_tensor(l2[:], hs[:], red[:, 0:1], op=Alu.subtract)
        nc.vector.tensor_tensor(red[:, 1:2], l2[:], ls[:], op=Alu.add)
        # cross-partition sum + broadcast in one exact fp32 ones-matmul:
        # out[m, j] = sum_p red[p, j] for every m
        pred = py.tile([128, 1024], f32, tag="ytile", name="pred")
        nc.tensor.matmul(pred[:, 0:2], ones128[:], red[:], start=True, stop=True)
        redo = statp.tile([128, 2], f32)
        nc.scalar.copy(redo[:], pred[:, 0:2])
        ssum = statp.tile([128, 1], f32)
        nc.vector.tensor_tensor(ssum[:], redo[:, 0:1], redo[:, 1:2], op=Alu.add)
        mean_t = statp.tile([128, 1], f32)
        nc.vector.tensor_scalar(mean_t[:], ssum[:], 1.0 / (DOUT * DIN), None, op0=Alu.mult)
        nc.vector.tensor_scalar(mean_t[:], mean_t[:], EPS, None, op0=Alu.max)
        s_w = statp.tile([128, 1], f32)  # 1/mean: the quantization scale
        nc.vector.reciprocal(s_w[:], mean_t[:])
        v_w = statp.tile([128, 1], f32)  # fl(1/s_w): the dequant magnitude (matches ref)
        nc.vector.reciprocal(v_w[:], s_w[:])
        vw127 = statp.tile([128, 1], f32)  # v_w / 127, folded once into the qs scale
        nc.vector.tensor_scalar(vw127[:], v_w[:], 1.0 / 127.0, None, op0=Alu.mult)

        # ---- W quantize + PE-transpose, in 4-chunk groups ----
        tT = tTp.tile([128, KC, DOUT], bf16)

        def quant_w_chunk(c):
            wc = w_all[:, c, :]
            wr1 = wtmpp.tile([128, DIN], f32, tag="wr1")
            nc.scalar.activation(wr1[:], wc, Copy, bias=MAGIC, scale=s_w[:])
            wr2 = wtmpp.tile([128, DIN], f32, tag="wr2")
            nc.vector.tensor_scalar(wr2[:], wr1[:], MAGIC, 1.0, op0=Alu.subtract, op1=Alu.min)
            wq = wqp.tile([128, DIN], bf16)
            nc.vector.tensor_scalar(wq[:], wr2[:], -1.0, None, op0=Alu.max)
            ptw = pt.tile([128, 512], bf16, tag="tp", name=f"ptw{c}")
            for k in range(KC):
                nc.tensor.transpose(
                    ptw[:, k * 128 : (k + 1) * 128], wq[:, k * 128 : (k + 1) * 128], ident[:]
                )
            # ptw[p, k*128+m] = t[c*128+m, k*128+p] -> tT[:, k, c*128+m]
            dst = tT[:, :, c * 128 : (c + 1) * 128]
            src = ptw[:].rearrange("p (k m) -> p k m", k=KC)
            if c % 2 == 0:
                nc.vector.tensor_copy(dst, src)
            else:
                nc.scalar.copy(dst, src)

        # ---- per-tile building blocks ----
        def frontend(i):
            xt = xts.pop(i) if i in xts else load_x(i)
            mx = mxp.tile([128, 1], f32, tag="mx")
            nc.vector.tensor_reduce(mx[:], xt[:], axis=X, op=Alu.max, apply_absolute_value=True)
            sx = mxp.tile([128, 1], f32, tag="sx")
            nc.vector.reciprocal(sx[:], mx[:])
            nc.vector.tensor_scalar(sx[:], sx[:], 127.0, None, op0=Alu.mult)
            c_tok = mxp.tile([128, 1], f32, tag="ct")
            nc.vector.tensor_tensor(c_tok[:], mx[:], vw127[:], op=Alu.mult)
            r1 = r1p.tile([128, DIN], f32)
            nc.scalar.activation(r1[:], xt[:], Copy, bias=MAGIC, scale=sx[:])
            # qs = (round(x*sx)) * c_tok, rounded once to bf16
            qs = qp.tile([128, DIN], bf16)
            nc.vector.tensor_scalar(qs[:], r1[:], MAGIC, c_tok[:], op0=Alu.subtract, op1=Alu.mult)
            return qs

        def transposes(qs):
            pq = pt.tile([128, 512], bf16, tag="tp", name="pq")
            for k in range(KC):
                nc.tensor.transpose(
                    pq[:, k * 128 : (k + 1) * 128], qs[:, k * 128 : (k + 1) * 128], ident[:]
                )
            qT = qtp.tile([128, KC, 128], bf16)
            nc.scalar.copy(qT[:], pq[:].rearrange("p (k m) -> p k m", k=KC))
            return qT

        def alloc_ph():
            return [py.tile([128, 1024], f32, tag="ytile", name="ph") for _ in range(2)]

        def mm_group(phs, qT, g):
            ph = phs[g // 2]
            nsl = (g % 2) * 512
            for k in range(KC):
                nc.tensor.matmul(
                    ph[:, nsl : nsl + 512], qT[:, k, :],
                    tT[:, k, g * 512 : (g + 1) * 512],
                    start=(k == 0), stop=(k == KC - 1),
                )

        def epilogue(j, phs):
            ysb = yp.tile([128, DOUT], bf16)
            for h in range(2):
                nc.vector.tensor_tensor(
                    ysb[:, h * 1024 : (h + 1) * 1024], phs[h][:],
                    bias_bc[:, h * 1024 : (h + 1) * 1024], op=Alu.add,
                )
            nc.sync.dma_start(y_d[j * 128 : (j + 1) * 128, :], ysb[:])

        # ---- startup: tile 0 interleaved with the W-quant groups ----
        qs_t = {0: frontend(0)}
        for c in range(0, 4):
            quant_w_chunk(c)
        qT_t = {0: transposes(qs_t.pop(0))}
        qs_t[1] = frontend(1)
        phs0 = alloc_ph()
        mm_group(phs0, qT_t[0], 0)
        for c in range(4, 8):
            quant_w_chunk(c)
        mm_group(phs0, qT_t[0], 1)
        qT_t[1] = transposes(qs_t.pop(1))
        qs_t[2] = frontend(2)
        for c in range(8, 12):
            quant_w_chunk(c)
        mm_group(phs0, qT_t[0], 2)
        for c in range(12, 16):
            quant_w_chunk(c)
        mm_group(phs0, qT_t[0], 3)
        del qT_t[0]
        state = {"phs": phs0}

        # ---- steady loop: fe 2 ahead, transposes 1 ahead, epilogue 1 behind ----
        for i in range(1, s_tiles):
            if i + 2 <= s_tiles - 1:
                qs_t[i + 2] = frontend(i + 2)
            if i + 1 <= s_tiles - 1:
                qT_t[i + 1] = transposes(qs_t.pop(i + 1))
            phs = alloc_ph()
            qT_i = qT_t.pop(i)
            for g in range(4):
                mm_group(phs, qT_i, g)
            epilogue(i - 1, state["phs"])
            state = {"phs": phs}

        epilogue(s_tiles - 1, state["phs"])

    nc.compile()
    return nc


def _get_program():
    if "nc" not in _cached:
        _cached["nc"] = build_program()
    return _cached["nc"]


def kernel(x: np.ndarray, weight: np.ndarray, bias: np.ndarray) -> np.ndarray:
    _ensure_path()
    from concourse.bass_utils import run_bass_kernel_spmd

    x = np.ascontiguousarray(x, dtype=np.float32)
    weight = np.ascontiguousarray(weight, dtype=np.float32)
    bias2d = np.ascontiguousarray(bias, dtype=np.float32).reshape(1, DOUT)

    nc = _get_program()
    in_maps = [
        {"x": x[c], "w": weight, "bias": bias2d} for c in range(N_CORES)
    ]
    res = run_bass_kernel_spmd(nc, in_maps, core_ids=list(range(N_CORES)))
    _cached["last_results"] = res
    y = np.stack(
        [np.asarray(res.results[c]["y"]).astype(np.float32) for c in range(N_CORES)],
        axis=0,
    )
    return y
